# revision 9
# baseline (speedup 1.0000x reference)
"""Trainium2 Bass kernel for the LP contrastive loss.

loss = mean_b( -log( pos_min_b / (pos_min_b + neg_sum_b + 1e-6) + 1e-6 ) )
  with E = exp(feats @ fs.T / TEMP), pos/neg split by label equality.

Strategy: shard the support set (N = Bs*TOPK = 16384) across the 8 cores
(2048 columns each); every core keeps the full query batch B = 2048.
Everything lives in SBUF (~19 MB/core in bf16), so after the initial load
the kernel is pure PE compute (~17 GFLOP/core in bf16).

Per core, for each 128x512 tile of the similarity matrix s = feats @ fs.T:
  v = s - 64 * (labels[b] == labels_s[n])        (mask via one DVE
      tensor_scalar is_equal*mult + one tensor_tensor add)
  row-min(v)  -> per-row min; positives sit at s-64 << negatives, so the
                 global min recovers min-over-positives of s.
  exp(20*v)   -> ScalarE activation with fused row-sum accumulation;
                 positive entries underflow to exactly 0, so the sum is
                 exactly the sum over negatives of exp(s/TEMP).
Host combines the 8 cores (min of mins, sum of sums) and applies the
final -log(...) / mean in float64.
"""

import sys

sys.path.insert(0, "/opt/trn_rl_repo")

import numpy as np
import ml_dtypes

TEMP = 0.05
SCALE = 1.0 / TEMP  # 20.0
BIG = 64.0
NCORES = 8

_CACHE = {}


def _build(B, C, Nsh, reps=1):
    """Build + compile the per-core Bass program (all cores run the same
    program on different data). reps>1 repeats the compute loop on-device
    (timing only -- lets the fixed dispatch overhead be subtracted out)."""
    import contextlib

    import concourse.tile as tile
    from concourse import bacc, mybir

    dt = mybir.dt
    MT = B // 128  # m-tiles (query rows)
    KT = C // 128  # k-tiles (contraction)
    NT = Nsh // 512  # n-tiles (support columns per core)

    nc = bacc.Bacc("TRN2", target_bir_lowering=False, debug=False, num_devices=NCORES)

    featsT = nc.dram_tensor("featsT", [C, B], dt.bfloat16, kind="ExternalInput").ap()
    fsT = nc.dram_tensor("fsT", [C, Nsh], dt.bfloat16, kind="ExternalInput").ap()
    lsb = nc.dram_tensor("lsb", [128, Nsh], dt.float32, kind="ExternalInput").ap()
    labm = nc.dram_tensor("labm", [128, MT], dt.float32, kind="ExternalInput").ap()
    minv_d = nc.dram_tensor("minv", [128, MT], dt.float32, kind="ExternalOutput").ap()
    sums_d = nc.dram_tensor("sums", [128, MT], dt.float32, kind="ExternalOutput").ap()

    CH = min(512, B)  # lhs chunk width -- keeps DMA lines >= 1KB/partition
    MC = B // CH  # lhs chunks

    with tile.TileContext(nc) as tc:
        with (
            tc.tile_pool(name="res", bufs=1) as res,
            tc.tile_pool(name="work", bufs=4) as work,
            tc.tile_pool(name="ps", bufs=8, space="PSUM") as psum,
        ):
            # --- resident tiles, DMA'd in the order compute consumes them ---
            # compute order: n-outer, m-inner, so the first sweep (n=0) needs
            # rhs[:, n0] + lhs chunks in m order; later n chunks arrive while
            # the PE is busy on earlier sweeps.
            labm_t = res.tile([128, MT], dt.float32, tag="labm")
            nc.sync.dma_start(labm_t[:], labm[:])
            ls_t = res.tile([128, Nsh], dt.float32, tag="lsb")
            nc.sync.dma_start(ls_t[:], lsb[:])

            lhs_tiles = [[None] * MC for _ in range(KT)]
            rhs_tiles = [[None] * NT for _ in range(KT)]
            for k in range(KT):
                lt = res.tile([128, CH], dt.bfloat16, tag=f"lhs{k}_0")
                nc.sync.dma_start(lt[:], featsT[k * 128 : (k + 1) * 128, 0:CH])
                lhs_tiles[k][0] = lt
                rt = res.tile([128, 512], dt.bfloat16, tag=f"rhs{k}_0")
                nc.sync.dma_start(rt[:], fsT[k * 128 : (k + 1) * 128, 0:512])
                rhs_tiles[k][0] = rt
            for c in range(1, MC):
                for k in range(KT):
                    lt = res.tile([128, CH], dt.bfloat16, tag=f"lhs{k}_{c}")
                    nc.sync.dma_start(
                        lt[:], featsT[k * 128 : (k + 1) * 128, c * CH : (c + 1) * CH]
                    )
                    lhs_tiles[k][c] = lt
            for n in range(1, NT):
                for k in range(KT):
                    rt = res.tile([128, 512], dt.bfloat16, tag=f"rhs{k}_{n}")
                    nc.sync.dma_start(
                        rt[:], fsT[k * 128 : (k + 1) * 128, n * 512 : (n + 1) * 512]
                    )
                    rhs_tiles[k][n] = rt

            mincols = res.tile([128, MT, NT], dt.float32, tag="mincols")
            sumcols = res.tile([128, MT, NT], dt.float32, tag="sumcols")
            minv_t = res.tile([128, MT], dt.float32, tag="minv")
            sums_t = res.tile([128, MT], dt.float32, tag="sums")

            rep_loop = (
                tc.For_i(
                    0,
                    reps,
                    1,
                    hint_engines=(mybir.EngineType.PE, mybir.EngineType.DVE),
                )
                if reps > 1
                else contextlib.nullcontext()
            )
            with rep_loop:
                for n in range(NT):
                    for m in range(MT):
                        c, ci = divmod(m * 128, CH)
                        ps = psum.tile([128, 512], dt.float32, tag="ps")
                        for k in range(KT):
                            nc.tensor.matmul(
                                ps[:],
                                lhs_tiles[k][c][:, ci : ci + 128],
                                rhs_tiles[k][n][:],
                                start=(k == 0),
                                stop=(k == KT - 1),
                            )
                        mask_t = work.tile([128, 512], dt.float32, tag="mask")
                        nc.vector.tensor_scalar(
                            mask_t[:],
                            ls_t[:, n * 512 : (n + 1) * 512],
                            labm_t[:, m : m + 1],
                            -BIG,
                            mybir.AluOpType.is_equal,
                            mybir.AluOpType.mult,
                        )
                        v_t = work.tile([128, 512], dt.float32, tag="v")
                        nc.vector.tensor_tensor(
                            v_t[:], ps[:], mask_t[:], mybir.AluOpType.add
                        )
                        nc.vector.tensor_reduce(
                            mincols[:, m, n : n + 1],
                            v_t[:],
                            axis=mybir.AxisListType.X,
                            op=mybir.AluOpType.min,
                        )
                        e_t = work.tile([128, 512], dt.float32, tag="e")
                        nc.scalar.activation(
                            e_t[:],
                            v_t[:],
                            mybir.ActivationFunctionType.Exp,
                            scale=SCALE,
                            accum_out=sumcols[:, m, n : n + 1],
                        )

            nc.vector.tensor_reduce(
                minv_t[:], mincols[:], axis=mybir.AxisListType.X, op=mybir.AluOpType.min
            )
            nc.vector.tensor_reduce(
                sums_t[:], sumcols[:], axis=mybir.AxisListType.X, op=mybir.AluOpType.add
            )
            nc.sync.dma_start(minv_d[:], minv_t[:])
            nc.sync.dma_start(sums_d[:], sums_t[:])

    nc.compile()
    return nc


def get_nc(B, C, Nsh, reps=1):
    key = (B, C, Nsh, reps)
    if key not in _CACHE:
        _CACHE[key] = _build(B, C, Nsh, reps)
    return _CACHE[key]


def make_in_maps(feats, feats_s, labels, labels_s):
    """Host-side prep: transpose/cast/shard the inputs for the 8 cores."""
    feats = np.asarray(feats, dtype=np.float32)
    fs = np.asarray(feats_s, dtype=np.float32).reshape(-1, feats.shape[1])
    labels = np.asarray(labels).astype(np.float32)
    labels_s = np.asarray(labels_s).astype(np.float32)

    B, C = feats.shape
    N = fs.shape[0]
    Nsh = N // NCORES
    MT = B // 128

    featsT = np.ascontiguousarray(feats.T).astype(ml_dtypes.bfloat16)
    # labels arranged so partition p, column t holds labels[t*128 + p]
    labm = np.ascontiguousarray(labels.reshape(MT, 128).T)

    in_maps = []
    for i in range(NCORES):
        fs_i = fs[i * Nsh : (i + 1) * Nsh]
        fsT_i = np.ascontiguousarray(fs_i.T).astype(ml_dtypes.bfloat16)
        ls_i = labels_s[i * Nsh : (i + 1) * Nsh]
        lsb_i = np.ascontiguousarray(np.broadcast_to(ls_i[None, :], (128, Nsh)))
        in_maps.append(
            {"featsT": featsT, "fsT": fsT_i, "lsb": lsb_i, "labm": labm}
        )
    return in_maps, B, C, Nsh


def finish_on_host(results, B):
    """Combine per-core partials into the scalar loss."""
    MT = B // 128
    minv = np.stack(
        [r["minv"].T.reshape(B) for r in results]
    )  # [NCORES, B], v-min per core
    sums = np.stack([r["sums"].T.reshape(B) for r in results])  # [NCORES, B]
    vmin = minv.min(axis=0).astype(np.float64)
    neg_sum = sums.astype(np.float64).sum(axis=0)
    # vmin = min_pos(s) - BIG  (positives are BIG below any negative sim)
    pos_min = np.exp(SCALE * vmin + SCALE * BIG)
    loss = -np.log(pos_min / (pos_min + neg_sum + 1e-6) + 1e-6)
    return np.float32(loss.mean())


def kernel(**inputs):
    from concourse.bass_utils import run_bass_kernel_spmd

    in_maps, B, C, Nsh = make_in_maps(
        inputs["feats"], inputs["feats_s"], inputs["labels"], inputs["labels_s"]
    )
    nc = get_nc(B, C, Nsh)
    res = run_bass_kernel_spmd(nc, in_maps, core_ids=list(range(NCORES)))
    return finish_on_host(res.results, B)


if __name__ == "__main__":
    rng = np.random.default_rng(0)
    B, C, Bs, TOPK = 2048, 2048, 4096, 4
    feats = rng.standard_normal((B, C), dtype=np.float32)
    feats /= np.linalg.norm(feats, axis=-1, keepdims=True)
    feats_s = rng.standard_normal((Bs, TOPK, C), dtype=np.float32)
    feats_s /= np.linalg.norm(feats_s, axis=-1, keepdims=True)
    labels = rng.integers(0, 256, B).astype(np.int32)
    labels_s = (np.arange(Bs * TOPK) % 256).astype(np.int32)
    out = kernel(feats=feats, feats_s=feats_s, labels=labels, labels_s=labels_s)
    print("loss:", out)


# revision 16
# speedup vs baseline: 6.6694x; 6.6694x over previous
"""Trainium2 Bass kernel for the LP contrastive loss.

loss = mean_b( -log( pos_min_b / (pos_min_b + neg_sum_b + 1e-6) + 1e-6 ) )
  with E = exp(feats @ fs.T / TEMP), pos/neg split by label equality.

Strategy: shard the support set (N = Bs*TOPK = 16384) across the 8 cores
(2048 columns each); every core keeps the full query batch B = 2048.
Everything lives in SBUF (~19 MB/core in bf16), so after the initial load
the kernel is pure PE compute (~17 GFLOP/core in bf16).

Per core, for each 128x512 tile of the similarity matrix s = feats @ fs.T:
  v = s - 64 * (labels[b] == labels_s[n])        (mask via one DVE
      tensor_scalar is_equal*mult + one tensor_tensor add)
  row-min(v)  -> per-row min; positives sit at s-64 << negatives, so the
                 global min recovers min-over-positives of s.
  exp(20*v)   -> ScalarE activation with fused row-sum accumulation;
                 positive entries underflow to exactly 0, so the sum is
                 exactly the sum over negatives of exp(s/TEMP).
Host combines the 8 cores (min of mins, sum of sums) and applies the
final -log(...) / mean in float64.
"""

import sys

sys.path.insert(0, "/opt/trn_rl_repo")

import numpy as np
import ml_dtypes

TEMP = 0.05
SCALE = 1.0 / TEMP  # 20.0
BIG = 64.0
NCORES = 8

_CACHE = {}


def _build(B, C, Nsh, reps=1):
    """Build + compile the per-core Bass program (all cores run the same
    program on different data). reps>1 repeats the compute loop on-device
    (timing only -- lets the fixed dispatch overhead be subtracted out)."""
    import contextlib

    import concourse.tile as tile
    from concourse import bacc, mybir

    dt = mybir.dt
    MT = B // 128  # m-tiles (query rows)
    KT = C // 128  # k-tiles (contraction)
    NT = Nsh // 512  # n-tiles (support columns per core)

    nc = bacc.Bacc("TRN2", target_bir_lowering=False, debug=False, num_devices=NCORES)

    featsT = nc.dram_tensor("featsT", [C, B], dt.bfloat16, kind="ExternalInput").ap()
    fsT = nc.dram_tensor("fsT", [C, Nsh], dt.bfloat16, kind="ExternalInput").ap()
    lsb = nc.dram_tensor("lsb", [128, Nsh], dt.float32, kind="ExternalInput").ap()
    labm = nc.dram_tensor("labm", [128, MT], dt.float32, kind="ExternalInput").ap()
    minv_d = nc.dram_tensor("minv", [128, MT], dt.float32, kind="ExternalOutput").ap()
    sums_d = nc.dram_tensor("sums", [128, MT], dt.float32, kind="ExternalOutput").ap()

    CH = min(512, B)  # lhs chunk width -- keeps DMA lines >= 1KB/partition
    MC = B // CH  # lhs chunks

    with tile.TileContext(nc) as tc:
        with (
            tc.tile_pool(name="res", bufs=1) as res,
            tc.tile_pool(name="work", bufs=4) as work,
            tc.tile_pool(name="ps", bufs=8, space="PSUM") as psum,
        ):
            # --- resident tiles, DMA'd in the order compute consumes them ---
            # compute order: n-outer, m-inner, so the first sweep (n=0) needs
            # rhs[:, n0] + lhs chunks in m order; later n chunks arrive while
            # the PE is busy on earlier sweeps.
            labm_t = res.tile([128, MT], dt.float32, tag="labm")
            nc.sync.dma_start(labm_t[:], labm[:])
            ls_t = res.tile([128, Nsh], dt.float32, tag="lsb")
            nc.sync.dma_start(ls_t[:], lsb[:])

            lhs_tiles = [[None] * MC for _ in range(KT)]
            rhs_tiles = [[None] * NT for _ in range(KT)]
            for k in range(KT):
                lt = res.tile([128, CH], dt.bfloat16, tag=f"lhs{k}_0")
                nc.sync.dma_start(lt[:], featsT[k * 128 : (k + 1) * 128, 0:CH])
                lhs_tiles[k][0] = lt
                rt = res.tile([128, 512], dt.bfloat16, tag=f"rhs{k}_0")
                nc.sync.dma_start(rt[:], fsT[k * 128 : (k + 1) * 128, 0:512])
                rhs_tiles[k][0] = rt
            for c in range(1, MC):
                for k in range(KT):
                    lt = res.tile([128, CH], dt.bfloat16, tag=f"lhs{k}_{c}")
                    nc.sync.dma_start(
                        lt[:], featsT[k * 128 : (k + 1) * 128, c * CH : (c + 1) * CH]
                    )
                    lhs_tiles[k][c] = lt
            for n in range(1, NT):
                for k in range(KT):
                    rt = res.tile([128, 512], dt.bfloat16, tag=f"rhs{k}_{n}")
                    nc.sync.dma_start(
                        rt[:], fsT[k * 128 : (k + 1) * 128, n * 512 : (n + 1) * 512]
                    )
                    rhs_tiles[k][n] = rt

            mincols = res.tile([128, MT, NT], dt.float32, tag="mincols")
            sumcols = res.tile([128, MT, NT], dt.float32, tag="sumcols")
            minv_t = res.tile([128, MT], dt.float32, tag="minv")
            sums_t = res.tile([128, MT], dt.float32, tag="sums")

            # PE warmup during the DMA prologue: ~30 dummy matmuls (~6 us)
            # keep the HAM activity window busy so the real matmuls start at
            # 2.4 GHz instead of ramping from 1.2 GHz. They depend only on a
            # memset tile, so they run while the input DMAs are in flight.
            warm = res.tile([128, 512], dt.bfloat16, tag="warm")
            nc.gpsimd.memset(warm[:], 0.0)
            wps = psum.tile([128, 512], dt.float32, tag="ps")
            for w in range(30):
                nc.tensor.matmul(
                    wps[:],
                    warm[:, 0:128],
                    warm[:],
                    start=(w == 0),
                    stop=(w == 29),
                )



            rep_loop = (
                tc.For_i(
                    0,
                    reps,
                    1,
                    hint_engines=(mybir.EngineType.PE, mybir.EngineType.DVE),
                )
                if reps > 1
                else contextlib.nullcontext()
            )
            with rep_loop:
                for n in range(NT):
                    for m in range(MT):
                        c, ci = divmod(m * 128, CH)
                        ps = psum.tile([128, 512], dt.float32, tag="ps")
                        for k in range(KT):
                            nc.tensor.matmul(
                                ps[:],
                                lhs_tiles[k][c][:, ci : ci + 128],
                                rhs_tiles[k][n][:],
                                start=(k == 0),
                                stop=(k == KT - 1),
                            )
                        mask_t = work.tile([128, 512], dt.float32, tag="mask")
                        nc.vector.tensor_scalar(
                            mask_t[:],
                            ls_t[:, n * 512 : (n + 1) * 512],
                            labm_t[:, m : m + 1],
                            -BIG,
                            mybir.AluOpType.is_equal,
                            mybir.AluOpType.mult,
                        )
                        v_t = work.tile([128, 512], dt.float32, tag="v")
                        nc.vector.tensor_tensor(
                            v_t[:], ps[:], mask_t[:], mybir.AluOpType.add
                        )
                        nc.vector.tensor_reduce(
                            mincols[:, m, n : n + 1],
                            v_t[:],
                            axis=mybir.AxisListType.X,
                            op=mybir.AluOpType.min,
                        )
                        e_t = work.tile([128, 512], dt.float32, tag="e")
                        nc.scalar.activation(
                            e_t[:],
                            v_t[:],
                            mybir.ActivationFunctionType.Exp,
                            scale=SCALE,
                            accum_out=sumcols[:, m, n : n + 1],
                        )

            nc.vector.tensor_reduce(
                minv_t[:], mincols[:], axis=mybir.AxisListType.X, op=mybir.AluOpType.min
            )
            nc.vector.tensor_reduce(
                sums_t[:], sumcols[:], axis=mybir.AxisListType.X, op=mybir.AluOpType.add
            )
            nc.sync.dma_start(minv_d[:], minv_t[:])
            nc.sync.dma_start(sums_d[:], sums_t[:])

    nc.compile()
    return nc


def get_nc(B, C, Nsh, reps=1):
    key = (B, C, Nsh, reps)
    if key not in _CACHE:
        _CACHE[key] = _build(B, C, Nsh, reps)
    return _CACHE[key]


def make_in_maps(feats, feats_s, labels, labels_s):
    """Host-side prep: transpose/cast/shard the inputs for the 8 cores."""
    feats = np.asarray(feats, dtype=np.float32)
    fs = np.asarray(feats_s, dtype=np.float32).reshape(-1, feats.shape[1])
    labels = np.asarray(labels).astype(np.float32)
    labels_s = np.asarray(labels_s).astype(np.float32)

    B, C = feats.shape
    N = fs.shape[0]
    Nsh = N // NCORES
    MT = B // 128

    featsT = np.ascontiguousarray(feats.T).astype(ml_dtypes.bfloat16)
    # labels arranged so partition p, column t holds labels[t*128 + p]
    labm = np.ascontiguousarray(labels.reshape(MT, 128).T)

    in_maps = []
    for i in range(NCORES):
        fs_i = fs[i * Nsh : (i + 1) * Nsh]
        fsT_i = np.ascontiguousarray(fs_i.T).astype(ml_dtypes.bfloat16)
        ls_i = labels_s[i * Nsh : (i + 1) * Nsh]
        lsb_i = np.ascontiguousarray(np.broadcast_to(ls_i[None, :], (128, Nsh)))
        in_maps.append(
            {"featsT": featsT, "fsT": fsT_i, "lsb": lsb_i, "labm": labm}
        )
    return in_maps, B, C, Nsh


def finish_on_host(results, B):
    """Combine per-core partials into the scalar loss."""
    MT = B // 128
    minv = np.stack(
        [r["minv"].T.reshape(B) for r in results]
    )  # [NCORES, B], v-min per core
    sums = np.stack([r["sums"].T.reshape(B) for r in results])  # [NCORES, B]
    vmin = minv.min(axis=0).astype(np.float64)
    neg_sum = sums.astype(np.float64).sum(axis=0)
    # vmin = min_pos(s) - BIG  (positives are BIG below any negative sim)
    pos_min = np.exp(SCALE * vmin + SCALE * BIG)
    loss = -np.log(pos_min / (pos_min + neg_sum + 1e-6) + 1e-6)
    return np.float32(loss.mean())


def kernel(**inputs):
    from concourse.bass_utils import run_bass_kernel_spmd

    in_maps, B, C, Nsh = make_in_maps(
        inputs["feats"], inputs["feats_s"], inputs["labels"], inputs["labels_s"]
    )
    nc = get_nc(B, C, Nsh)
    res = run_bass_kernel_spmd(nc, in_maps, core_ids=list(range(NCORES)))
    return finish_on_host(res.results, B)


if __name__ == "__main__":
    rng = np.random.default_rng(0)
    B, C, Bs, TOPK = 2048, 2048, 4096, 4
    feats = rng.standard_normal((B, C), dtype=np.float32)
    feats /= np.linalg.norm(feats, axis=-1, keepdims=True)
    feats_s = rng.standard_normal((Bs, TOPK, C), dtype=np.float32)
    feats_s /= np.linalg.norm(feats_s, axis=-1, keepdims=True)
    labels = rng.integers(0, 256, B).astype(np.int32)
    labels_s = (np.arange(Bs * TOPK) % 256).astype(np.int32)
    out = kernel(feats=feats, feats_s=feats_s, labels=labels, labels_s=labels_s)
    print("loss:", out)


# revision 21
# speedup vs baseline: 6.7184x; 1.0074x over previous
"""Trainium2 Bass kernel for the LP contrastive loss.

loss = mean_b( -log( pos_min_b / (pos_min_b + neg_sum_b + 1e-6) + 1e-6 ) )
  with E = exp(feats @ fs.T / TEMP), pos/neg split by label equality.

Strategy: shard the support set (N = Bs*TOPK = 16384) across the 8 cores
(2048 columns each); every core keeps the full query batch B = 2048.
Everything lives in SBUF (~19 MB/core in bf16), so after the initial load
the kernel is pure PE compute (~17 GFLOP/core in bf16).

Per core, for each 128x512 tile of the similarity matrix s = feats @ fs.T:
  v = s - 64 * (labels[b] == labels_s[n])        (mask via one DVE
      tensor_scalar is_equal*mult + one tensor_tensor add)
  row-min(v)  -> per-row min; positives sit at s-64 << negatives, so the
                 global min recovers min-over-positives of s.
  exp(20*v)   -> ScalarE activation with fused row-sum accumulation;
                 positive entries underflow to exactly 0, so the sum is
                 exactly the sum over negatives of exp(s/TEMP).
Host combines the 8 cores (min of mins, sum of sums) and applies the
final -log(...) / mean in float64.
"""

import sys

sys.path.insert(0, "/opt/trn_rl_repo")

import numpy as np
import ml_dtypes

TEMP = 0.05
SCALE = 1.0 / TEMP  # 20.0
BIG = 64.0
NCORES = 8

_CACHE = {}


def _build(B, C, Nsh, reps=1):
    """Build + compile the per-core Bass program (all cores run the same
    program on different data). reps>1 repeats the compute loop on-device
    (timing only -- lets the fixed dispatch overhead be subtracted out)."""
    import contextlib

    import concourse.tile as tile
    from concourse import bacc, mybir

    dt = mybir.dt
    MT = B // 128  # m-tiles (query rows)
    KT = C // 128  # k-tiles (contraction)
    NT = Nsh // 512  # n-tiles (support columns per core)

    nc = bacc.Bacc("TRN2", target_bir_lowering=False, debug=False, num_devices=NCORES)

    featsT = nc.dram_tensor("featsT", [C, B], dt.bfloat16, kind="ExternalInput").ap()
    fsT = nc.dram_tensor("fsT", [C, Nsh], dt.bfloat16, kind="ExternalInput").ap()
    lsb = nc.dram_tensor("lsb", [128, Nsh], dt.float32, kind="ExternalInput").ap()
    labm = nc.dram_tensor("labm", [128, MT], dt.float32, kind="ExternalInput").ap()
    # per-partition copy of -BIG (runtime-chosen mask offset)
    bigv = nc.dram_tensor("bigv", [128, 1], dt.float32, kind="ExternalInput").ap()
    minv_d = nc.dram_tensor("minv", [128, MT], dt.float32, kind="ExternalOutput").ap()
    sums_d = nc.dram_tensor("sums", [128, MT], dt.float32, kind="ExternalOutput").ap()

    CH = min(512, B)  # lhs chunk width -- keeps DMA lines >= 1KB/partition
    MC = B // CH  # lhs chunks

    with tile.TileContext(nc) as tc:
        with (
            tc.tile_pool(name="res", bufs=1) as res,
            tc.tile_pool(name="work", bufs=4) as work,
            tc.tile_pool(name="ps", bufs=8, space="PSUM") as psum,
        ):
            # --- resident tiles, DMA'd in the order compute consumes them ---
            # compute order: n-outer, m-inner, so the first sweep (n=0) needs
            # rhs[:, n0] + lhs chunks in m order; later n chunks arrive while
            # the PE is busy on earlier sweeps.
            labm_t = res.tile([128, MT], dt.float32, tag="labm")
            nc.sync.dma_start(labm_t[:], labm[:])
            bigv_t = res.tile([128, 1], dt.float32, tag="bigv")
            nc.sync.dma_start(bigv_t[:], bigv[:])
            ls_t = res.tile([128, Nsh], dt.float32, tag="lsb")
            nc.sync.dma_start(ls_t[:], lsb[:])

            lhs_tiles = [[None] * MC for _ in range(KT)]
            rhs_tiles = [[None] * NT for _ in range(KT)]
            for k in range(KT):
                lt = res.tile([128, CH], dt.bfloat16, tag=f"lhs{k}_0")
                nc.sync.dma_start(lt[:], featsT[k * 128 : (k + 1) * 128, 0:CH])
                lhs_tiles[k][0] = lt
                rt = res.tile([128, 512], dt.bfloat16, tag=f"rhs{k}_0")
                nc.sync.dma_start(rt[:], fsT[k * 128 : (k + 1) * 128, 0:512])
                rhs_tiles[k][0] = rt
            for c in range(1, MC):
                for k in range(KT):
                    lt = res.tile([128, CH], dt.bfloat16, tag=f"lhs{k}_{c}")
                    nc.sync.dma_start(
                        lt[:], featsT[k * 128 : (k + 1) * 128, c * CH : (c + 1) * CH]
                    )
                    lhs_tiles[k][c] = lt
            for n in range(1, NT):
                for k in range(KT):
                    rt = res.tile([128, 512], dt.bfloat16, tag=f"rhs{k}_{n}")
                    nc.sync.dma_start(
                        rt[:], fsT[k * 128 : (k + 1) * 128, n * 512 : (n + 1) * 512]
                    )
                    rhs_tiles[k][n] = rt

            mincols = res.tile([128, MT, NT], dt.float32, tag="mincols")
            sumcols = res.tile([128, MT, NT], dt.float32, tag="sumcols")
            minv_t = res.tile([128, MT], dt.float32, tag="minv")
            sums_t = res.tile([128, MT], dt.float32, tag="sums")

            # PE warmup during the DMA prologue: ~30 dummy matmuls (~6 us)
            # keep the HAM activity window busy so the real matmuls start at
            # 2.4 GHz instead of ramping from 1.2 GHz. They depend only on a
            # memset tile, so they run while the input DMAs are in flight.
            warm = res.tile([128, 512], dt.bfloat16, tag="warm")
            nc.gpsimd.memset(warm[:], 0.0)
            wps = psum.tile([128, 512], dt.float32, tag="ps")
            for w in range(30):
                nc.tensor.matmul(
                    wps[:],
                    warm[:, 0:128],
                    warm[:],
                    start=(w == 0),
                    stop=(w == 29),
                )



            rep_loop = (
                tc.For_i(
                    0,
                    reps,
                    1,
                    hint_engines=(mybir.EngineType.PE, mybir.EngineType.DVE),
                )
                if reps > 1
                else contextlib.nullcontext()
            )
            with rep_loop:
                for n in range(NT):
                    for m in range(MT):
                        c, ci = divmod(m * 128, CH)
                        ps = psum.tile([128, 512], dt.float32, tag="ps")
                        for k in range(KT):
                            nc.tensor.matmul(
                                ps[:],
                                lhs_tiles[k][c][:, ci : ci + 128],
                                rhs_tiles[k][n][:],
                                start=(k == 0),
                                stop=(k == KT - 1),
                            )
                        mask_t = work.tile([128, 512], dt.float32, tag="mask")
                        nc.vector.tensor_scalar(
                            mask_t[:],
                            ls_t[:, n * 512 : (n + 1) * 512],
                            labm_t[:, m : m + 1],
                            bigv_t[:, 0:1],
                            mybir.AluOpType.is_equal,
                            mybir.AluOpType.mult,
                        )
                        v_t = work.tile([128, 512], dt.float32, tag="v")
                        nc.vector.tensor_tensor(
                            v_t[:], ps[:], mask_t[:], mybir.AluOpType.add
                        )
                        nc.vector.tensor_reduce(
                            mincols[:, m, n : n + 1],
                            v_t[:],
                            axis=mybir.AxisListType.X,
                            op=mybir.AluOpType.min,
                        )
                        e_t = work.tile([128, 512], dt.float32, tag="e")
                        nc.scalar.activation(
                            e_t[:],
                            v_t[:],
                            mybir.ActivationFunctionType.Exp,
                            scale=SCALE,
                            accum_out=sumcols[:, m, n : n + 1],
                        )

            nc.vector.tensor_reduce(
                minv_t[:], mincols[:], axis=mybir.AxisListType.X, op=mybir.AluOpType.min
            )
            nc.vector.tensor_reduce(
                sums_t[:], sumcols[:], axis=mybir.AxisListType.X, op=mybir.AluOpType.add
            )
            nc.sync.dma_start(minv_d[:], minv_t[:])
            nc.sync.dma_start(sums_d[:], sums_t[:])

    nc.compile()
    return nc


def get_nc(B, C, Nsh, reps=1):
    key = (B, C, Nsh, reps)
    if key not in _CACHE:
        _CACHE[key] = _build(B, C, Nsh, reps)
    return _CACHE[key]


def make_in_maps(feats, feats_s, labels, labels_s):
    """Host-side prep: transpose/cast/shard the inputs for the 8 cores."""
    feats = np.asarray(feats, dtype=np.float32)
    fs = np.asarray(feats_s, dtype=np.float32).reshape(-1, feats.shape[1])
    labels = np.asarray(labels).astype(np.float32)
    labels_s = np.asarray(labels_s).astype(np.float32)

    B, C = feats.shape
    N = fs.shape[0]
    Nsh = N // NCORES
    MT = B // 128

    featsT = np.ascontiguousarray(feats.T).astype(ml_dtypes.bfloat16)
    # labels arranged so partition p, column t holds labels[t*128 + p]
    labm = np.ascontiguousarray(labels.reshape(MT, 128).T)

    # mask offset: must exceed the sim range so positives (s - big) always
    # sit below any negative sim. |s| <= max||feats_b|| * max||fs_n||; for
    # l2-normalized inputs the bound is 1 and big stays at the default 64.
    bound = float(
        np.linalg.norm(feats, axis=1).max() * np.linalg.norm(fs, axis=1).max()
    )
    big = max(BIG, 4.0 * bound)
    bigv = np.full((128, 1), -big, np.float32)

    in_maps = []
    for i in range(NCORES):
        fs_i = fs[i * Nsh : (i + 1) * Nsh]
        fsT_i = np.ascontiguousarray(fs_i.T).astype(ml_dtypes.bfloat16)
        ls_i = labels_s[i * Nsh : (i + 1) * Nsh]
        lsb_i = np.ascontiguousarray(np.broadcast_to(ls_i[None, :], (128, Nsh)))
        in_maps.append(
            {"featsT": featsT, "fsT": fsT_i, "lsb": lsb_i, "labm": labm, "bigv": bigv}
        )
    return in_maps, B, C, Nsh, big


def finish_on_host(results, B, big=BIG):
    """Combine per-core partials into the scalar loss."""
    MT = B // 128
    minv = np.stack(
        [r["minv"].T.reshape(B) for r in results]
    )  # [NCORES, B], v-min per core
    sums = np.stack([r["sums"].T.reshape(B) for r in results])  # [NCORES, B]
    vmin = minv.min(axis=0).astype(np.float64)
    neg_sum = sums.astype(np.float64).sum(axis=0)
    # vmin = min_pos(s) - big  (positives sit big below any negative sim)
    with np.errstate(over="ignore", invalid="ignore"):
        pos_min = np.exp(SCALE * vmin + SCALE * big)
        loss = -np.log(pos_min / (pos_min + neg_sum + 1e-6) + 1e-6)
    return np.float32(loss.mean())


def kernel(**inputs):
    from concourse.bass_utils import run_bass_kernel_spmd

    in_maps, B, C, Nsh, big = make_in_maps(
        inputs["feats"], inputs["feats_s"], inputs["labels"], inputs["labels_s"]
    )
    nc = get_nc(B, C, Nsh)
    res = run_bass_kernel_spmd(nc, in_maps, core_ids=list(range(NCORES)))
    return finish_on_host(res.results, B, big)


if __name__ == "__main__":
    rng = np.random.default_rng(0)
    B, C, Bs, TOPK = 2048, 2048, 4096, 4
    feats = rng.standard_normal((B, C), dtype=np.float32)
    feats /= np.linalg.norm(feats, axis=-1, keepdims=True)
    feats_s = rng.standard_normal((Bs, TOPK, C), dtype=np.float32)
    feats_s /= np.linalg.norm(feats_s, axis=-1, keepdims=True)
    labels = rng.integers(0, 256, B).astype(np.int32)
    labels_s = (np.arange(Bs * TOPK) % 256).astype(np.int32)
    out = kernel(feats=feats, feats_s=feats_s, labels=labels, labels_s=labels_s)
    print("loss:", out)
